# revision 14
# baseline (speedup 1.0000x reference)
"""Trainium2 Bass kernel for nn_Encoding (VQ codebook soft-assignment encoding).

Reference computation (per batch b, with n = H*W pixels):
    xr[n, d]   = x[b].reshape(D, N).T
    sl[n, k]   = scale_k^2 * (||xr_n||^2 - 2 xr_n.c_k + ||c_k||^2)
    a[n, k]    = softmax_k(sl)
    e[b, k, d] = sum_n a[n,k] * xr[n,d]  -  (sum_n a[n,k]) * c[k,d]

Sharding: data-parallel over batch: 16 batches -> 8 cores x 2 batches.
Codewords/scale replicated. No collectives needed.

Device mapping per core (B_PER_CORE=2, D=512, N=4096, K=32):
  - x stays in its natural [d, n] layout for the first matmul:
      psum_lin[n, k] = sum_d x[d, n] * cts[d, k]   (cts = -2*s2_k*c_k, so this
      is the -2*s2*x.c term directly), x as the PE stationary, f32r.
  - x2 = sum_d x^2 lands in the same psum tile (column K) via a second
      accumulation group: lhsT = xsq tile (ACT Square), rhs = ones column.
  - softmax runs along the free axis on [128, 4, 33] psum tiles (n on
      partitions, k innermost) on DVE + one ACT Exp.
  - The second matmul contracts n, which needs x transposed: PE transpose
      (matmul with identity) -> psum -> SBUF, then
      psum_e[k, d] += a_tile[n, k].T @ xT_tile[n, d] accumulated over all n.
  - e = psum_e - asum[k] * c[k, d], asum accumulated with a ones matmul.
"""

import numpy as np

import concourse.bass as bass
import concourse.bacc as bacc
import concourse.mybir as mybir
from concourse import tile

F32 = mybir.dt.float32
F32R = mybir.dt.float32r
AF = mybir.ActivationFunctionType
AX = mybir.AxisListType
ALU = mybir.AluOpType

B, D, H, W, K = 16, 512, 64, 64, 32
N = H * W                    # 4096 pixels per batch
NCORES = 8
BPC = B // NCORES            # 2 batches per core
DC = D // 128                # 4 contraction chunks
NG = 8                       # n-groups of 512 per batch
NSUB = 4                     # 128-pixel subtiles per group


def f32(ap):
    """Read a float32r-typed access pattern as plain fp32 (same bytes)."""
    return ap.bitcast(F32)


def build_nc() -> bass.Bass:
    # fp32 matmuls on TRN2 run at 4 cycles/row; float32r (same 4-byte layout,
    # TF32-class rounding inside the PE, ~2.4e-4 median rel err measured on
    # hw) streams at 1 cycle/row for free dims >= 256. Everything that feeds
    # the PE is therefore declared float32r; DVE/ACT read those tiles
    # bitcast back to fp32.
    nc = bacc.Bacc("TRN2", target_bir_lowering=False, debug=False,
                   num_devices=NCORES)

    x = nc.dram_tensor("x", [BPC, D, N], F32R, kind="ExternalInput").ap()
    cts = nc.dram_tensor("cts", [D, K], F32R, kind="ExternalInput").ap()
    c_kd = nc.dram_tensor("c_kd", [K, D], F32, kind="ExternalInput").ap()
    s2rep = nc.dram_tensor("s2rep", [128, K], F32, kind="ExternalInput").ap()
    c2s2rep = nc.dram_tensor("c2s2rep", [128, K], F32, kind="ExternalInput").ap()
    ones = nc.dram_tensor("ones", [128, 2], F32R, kind="ExternalInput").ap()
    ident = nc.dram_tensor("ident", [128, 128], F32R, kind="ExternalInput").ap()
    e = nc.dram_tensor("e", [BPC, K, D], F32, kind="ExternalOutput").ap()

    from contextlib import ExitStack
    with tile.TileContext(nc) as tc, ExitStack() as ctx:
        const = ctx.enter_context(tc.tile_pool(name="const", bufs=1))
        xpool = ctx.enter_context(tc.tile_pool(name="x", bufs=2))
        sqpool = ctx.enter_context(tc.tile_pool(name="xsq", bufs=2))
        smpool = ctx.enter_context(tc.tile_pool(name="softmax", bufs=3))
        xtpool = ctx.enter_context(tc.tile_pool(name="xt", bufs=3))
        outpool = ctx.enter_context(tc.tile_pool(name="out", bufs=2))
        ps_sl = ctx.enter_context(tc.tile_pool(name="ps_sl", bufs=2, space="PSUM"))
        ps_xt = ctx.enter_context(tc.tile_pool(name="ps_xt", bufs=2, space="PSUM"))
        ps_e = ctx.enter_context(tc.tile_pool(name="ps_e", bufs=2, space="PSUM"))
        ps_as = ctx.enter_context(tc.tile_pool(name="ps_as", bufs=2, space="PSUM"))

        # Constants, loaded once.
        cts_sb = const.tile([128, DC, K], F32R)
        for c in range(DC):
            nc.sync.dma_start(out=cts_sb[:, c, :], in_=cts[c * 128:(c + 1) * 128, :])
        ckd_sb = const.tile([K, D], F32)
        nc.sync.dma_start(out=ckd_sb[:], in_=c_kd[:])
        s2_sb = const.tile([128, K], F32)
        nc.sync.dma_start(out=s2_sb[:], in_=s2rep[:])
        c2s2_sb = const.tile([128, K], F32)
        nc.sync.dma_start(out=c2s2_sb[:], in_=c2s2rep[:])
        ones_sb = const.tile([128, 2], F32R)
        nc.sync.dma_start(out=ones_sb[:], in_=ones[:])
        id_sb = const.tile([128, 128], F32R)
        nc.sync.dma_start(out=id_sb[:], in_=ident[:])

        for b in range(BPC):
            psum_e = ps_e.tile([K, D], F32)
            psum_as = ps_as.tile([K, 2], F32)
            for g in range(NG):
                n0 = g * 512
                first_g, last_g = (g == 0), (g == NG - 1)

                # ---- load one n-group of x: [128, DC, 512] ([d, n]) ----
                xg = xpool.tile([128, DC, 512], F32R)
                for c in range(DC):
                    nc.sync.dma_start(
                        out=xg[:, c, :],
                        in_=x[b, c * 128:(c + 1) * 128, n0:n0 + 512])

                # ---- xsq for the x2 column ----
                xsq = sqpool.tile([128, DC, 512], F32R)
                nc.scalar.activation(xsq[:], f32(xg[:]), AF.Square)

                # ---- logits: psum_sl[:, j, 0:K] = -2 s2 (x.c); [..., K] = x2
                psum_sl = ps_sl.tile([128, NSUB, K + 2], F32)
                for j in range(NSUB):
                    js = slice(j * 128, (j + 1) * 128)
                    for c in range(DC):
                        nc.tensor.matmul(
                            psum_sl[:, j, 0:K],
                            lhsT=xg[:, c, js], rhs=cts_sb[:, c, :],
                            start=(c == 0), stop=(c == DC - 1))
                    for c in range(DC):
                        nc.tensor.matmul(
                            psum_sl[:, j, K:K + 2],
                            lhsT=xsq[:, c, js], rhs=ones_sb[:],
                            start=(c == 0), stop=(c == DC - 1))

                # ---- softmax over k (free axis) ----
                # sl = lin + s2*x2 + s2*c2   (all [128, NSUB, K] views)
                x2b = psum_sl[:, :, K:K + 1].broadcast_to([128, NSUB, K])
                s2b = s2_sb[:, None, :].broadcast_to([128, NSUB, K])
                c2b = c2s2_sb[:, None, :].broadcast_to([128, NSUB, K])
                t1 = smpool.tile([128, NSUB, K], F32, tag="t1")
                nc.vector.tensor_tensor(t1[:], x2b, s2b, ALU.mult)
                sl = smpool.tile([128, NSUB, K], F32, tag="sl")
                nc.vector.tensor_tensor(sl[:], psum_sl[:, :, 0:K], t1[:], ALU.add)
                sl2 = smpool.tile([128, NSUB, K], F32, tag="sl2")
                nc.vector.tensor_tensor(sl2[:], sl[:], c2b, ALU.add)
                nm = smpool.tile([128, NSUB], F32, tag="nm")
                nc.vector.tensor_reduce(nm[:], sl2[:], AX.X, ALU.max, negate=True)
                es = smpool.tile([128, NSUB, K], F32, tag="es")
                nmb = nm[:, :, None].broadcast_to([128, NSUB, K])
                nc.vector.tensor_tensor(es[:], sl2[:], nmb, ALU.add)
                p = smpool.tile([128, NSUB, K], F32, tag="p")
                nc.scalar.activation(p[:], es[:], AF.Exp)
                s = smpool.tile([128, NSUB], F32, tag="s")
                nc.vector.tensor_reduce(s[:], p[:], AX.X, ALU.add)
                rec = smpool.tile([128, NSUB], F32, tag="rec")
                nc.vector.reciprocal(rec[:], s[:])
                a = smpool.tile([128, NSUB, K], F32R, tag="a")
                recb = rec[:, :, None].broadcast_to([128, NSUB, K])
                nc.vector.tensor_tensor(a[:], p[:], recb, ALU.mult)

                # ---- asum[k] += sum_n a ----
                for j in range(NSUB):
                    nc.tensor.matmul(
                        psum_as[:], lhsT=a[:, j, :], rhs=ones_sb[:],
                        start=(first_g and j == 0), stop=(last_g and j == NSUB - 1),
                        skip_group_check=True)

                # ---- transpose x, then e[k, d] += a.T @ xT ----
                for j in range(NSUB):
                    js = slice(j * 128, (j + 1) * 128)
                    psum_xt = ps_xt.tile([128, D], F32R)
                    for c in range(DC):
                        nc.tensor.transpose(
                            psum_xt[:, c * 128:(c + 1) * 128],
                            xg[:, c, js], id_sb[:])
                    xt = xtpool.tile([128, D], F32R)
                    nc.scalar.activation(xt[:], f32(psum_xt[:]), AF.Copy)
                    nc.tensor.matmul(
                        psum_e[:], lhsT=a[:, j, :], rhs=xt[:],
                        start=(first_g and j == 0), stop=(last_g and j == NSUB - 1),
                        skip_group_check=True)

            # ---- e = psum_e - asum * c ----
            asb = psum_as[:, 0:1].broadcast_to([K, D])
            tmp = outpool.tile([K, D], F32, tag="tmp")
            nc.vector.tensor_tensor(tmp[:], asb, ckd_sb[:], ALU.mult)
            e_sb = outpool.tile([K, D], F32, tag="e_sb")
            nc.vector.tensor_tensor(e_sb[:], psum_e[:], tmp[:], ALU.subtract)
            nc.sync.dma_start(out=e[b], in_=e_sb[:])

    nc.compile()
    return nc


_NC_CACHE = None


def get_nc() -> bass.Bass:
    global _NC_CACHE
    if _NC_CACHE is None:
        _NC_CACHE = build_nc()
    return _NC_CACHE


def make_in_maps(x, codewords, scale):
    assert x.shape == (B, D, H, W) and codewords.shape == (K, D)
    x = np.ascontiguousarray(x, dtype=np.float32).reshape(B, D, N)
    codewords = np.ascontiguousarray(codewords, dtype=np.float32)
    scale = np.ascontiguousarray(scale, dtype=np.float32)

    s2 = scale * scale                                   # [K]
    c2 = (codewords * codewords).sum(axis=1)             # [K]
    cts = (-2.0 * s2[:, None] * codewords).T.copy()      # [D, K]
    s2rep = np.broadcast_to(s2, (128, K)).copy()
    c2s2rep = np.broadcast_to(s2 * c2, (128, K)).copy()
    ones = np.ones((128, 2), np.float32)
    ident = np.eye(128, dtype=np.float32)

    in_maps = []
    for i in range(NCORES):
        in_maps.append({
            "x": np.ascontiguousarray(x[i * BPC:(i + 1) * BPC]),
            "cts": cts, "c_kd": codewords,
            "s2rep": s2rep, "c2s2rep": c2s2rep,
            "ones": ones, "ident": ident,
        })
    return in_maps


def kernel(x: np.ndarray, codewords: np.ndarray, scale: np.ndarray) -> np.ndarray:
    from concourse.bass_utils import run_bass_kernel_spmd

    in_maps = make_in_maps(x, codewords, scale)
    res = run_bass_kernel_spmd(get_nc(), in_maps, list(range(NCORES)))
    return np.concatenate([res.results[i]["e"] for i in range(NCORES)], axis=0)
